# revision 37
# baseline (speedup 1.0000x reference)
"""Trainium2 Bass kernel for nn_MixtureExpertsMlp (MoE soft routing).

Contract: kernel(**inputs) takes the FULL unsharded inputs
(x [4,4096,768], phi [4,1024,768], w1 [4,768,3072], b1 [4,3072],
w2 [4,3072,768], b2 [4,768]) and returns the FULL output [4,4096,768].

Sharding (expert+slot parallel over 8 NeuronCores): core c owns expert
e = c // 2 and slot half h = c % 2, i.e. SL = 512 of that expert's 1024
routing slots. Every core sees all tokens. Per core and per batch b:

  L^T[s, n]    = sum_d phi[s, d] x[b, n, d]        (slots on partitions)
  E^T          = exp(L^T)          (softmax max-subtraction skipped: the
                                    logits are ~N(0,1), well within fp32)
  ddenom[s]    = sum_n E^T[s, n]                    (via ACT accum_out)
  D[n, s]      = E^T[s, n] / ddenom[s]    (dispatch; transposed+normalized
                                    per 128-slot block in one matmul
                                    against diag(1/ddenom) blocks)
  slots^T[d,s] = sum_n x[b, n, d] D[n, s]
  h^T[h', s]   = gelu_tanh(sum_d w1[d, h'] slots^T[d, s] + b1[h'])
  y[s, d]      = sum_h h^T[h, s]^T w2[h, d]   (directly in [s, d] layout:
                                    stationary = h^T chunk, moving = w2 row)
  outp[n, :D]  = sum_s E^T[s, n] y[s, :]      (unnormalized combine)
  outp[n, D]   = sum_s E^T[s, n]              (ones column appended to y)

Host-side unshard: the combine softmax normalizer is global over all
E*S slots, so out = (sum_c num_c + sum_c gdl_c * b2[e(c)]) / sum_c gdl_c
where num_c = outp_c[..., :D] and gdl_c = outp_c[..., D]. This also
folds in b2 exactly (per-expert combine mass times b2[e]).

Precision: logits matmul in float32r (exp amplifies logit error into
routing-weight relative error). Everything downstream (dispatch
weights, slots, MLP, combine) in bf16 — ~1e-3 relative noise against a
2e-2 gate. w1/w2 live in SBUF in bf16 for the whole kernel (loaded
once, not per batch); outputs are written bf16 and combined on host in
float64.
"""

import numpy as np
from contextlib import ExitStack

import ml_dtypes

import concourse.bass as bass
import concourse.tile as tile
from concourse import mybir
from concourse.bass import ts
from concourse.masks import make_identity
from concourse.bass_utils import run_bass_kernel_spmd

F32 = mybir.dt.float32
F32R = mybir.dt.float32r
BF16 = mybir.dt.bfloat16
F8 = mybir.dt.float8e4
AF = mybir.ActivationFunctionType
DR = mybir.MatmulPerfMode.DoubleRow
BF = ml_dtypes.bfloat16
F8NP = ml_dtypes.float8_e4m3  # TRN e4m3: max +-240, inf beyond — matches HW

N_CORES = 8
# fp8 logits scales: logits_psum = (SX*x) @ (SPHI*phi); exp() applies 1/(SX*SPHI).
SX = 16.0
SPHI = 512.0
# fp8 combine scales: eT stored as 2^-4*exp(L) (via the exp bias), y stored as
# SY*y, ones column stored as SONE. Host unscales consistently.
SE_LOG2 = -4
SY = 256.0
SONE = 0.125
import math

EXP_BIAS = SE_LOG2 * math.log(2.0)


# --------------------------------------------------------------------------
# Post-pass: the walrus build in this container enforces the ISA cap of one
# sync-wait per instruction (two for EventSemaphore); Tile's final drain can
# carry more. Hoist excess waits onto fresh same-engine NOPs.
# --------------------------------------------------------------------------
def _split_excess_waits(nc):
    caps = {"InstEventSemaphore": 2}
    n_new = 0
    for f in nc.m.functions:
        for bb in f.blocks:
            i = 0
            insts = bb.instructions
            while i < len(insts):
                ins = insts[i]
                si = ins.sync_info
                cap = caps.get(type(ins).__name__, 1)
                if si is not None and len(si.on_wait) > cap:
                    waits = list(si.on_wait)
                    keep, hoist = waits[-cap:], waits[:-cap]
                    new_nops = []
                    for w in hoist:
                        nop = mybir.InstNoOp(
                            name=nc.get_next_instruction_name(),
                            engine=ins.engine,
                            ins=[],
                            outs=[],
                            sync_info=mybir.SyncInfo(on_wait=[w], on_update=[]),
                        )
                        nc.register_instruction(nop)
                        new_nops.append(nop)
                    ins.sync_info = mybir.SyncInfo(
                        on_wait=keep, on_update=list(si.on_update)
                    )
                    insts[i:i] = new_nops
                    i += len(new_nops)
                    n_new += len(new_nops)
                i += 1
    return n_new


def _emit_moe_kernel(nc, B, N, D, SL, H, act_fn=AF.Gelu_apprx_tanh):
    assert N % 512 == 0 and D % 128 == 0 and SL % 128 == 0 and H % 128 == 0
    Dc, SLc, Hc = D // 128, SL // 128, H // 128
    NT, NV = N // 512, N // 128
    OD = D + 2  # output: D columns + ones column (combine denom) + pad
    ODP = D + 16  # y_aug width: DoubleRow needs the pair step % 16 == 0

    # All inputs are pre-arranged host-side to partition-major layouts so
    # every DMA is one contiguous run per partition (few descriptors).
    xT = nc.dram_tensor("xT", [B, NT, 128, Dc, 512], F8, kind="ExternalInput").ap()
    xnb = nc.dram_tensor("xnb", [B, N, D], BF16, kind="ExternalInput").ap()
    phiT = nc.dram_tensor("phiT", [128, Dc, SL], F8, kind="ExternalInput").ap()
    w1b = nc.dram_tensor("w1b", [128, Dc, H], BF16, kind="ExternalInput").ap()
    w2b = nc.dram_tensor("w2b", [128, Hc, D], BF16, kind="ExternalInput").ap()
    b1 = nc.dram_tensor("b1", [128, Hc], F32, kind="ExternalInput").ap()
    outp = nc.dram_tensor("outp", [B, N, OD], BF16, kind="ExternalOutput").ap()

    with tile.TileContext(nc) as tc, ExitStack() as ctx:
        pool = lambda name, bufs, space="SBUF": ctx.enter_context(
            tc.tile_pool(name=name, bufs=bufs, space=space)
        )
        singles = pool("singles", 1)
        eT_pool = pool("eT", 1)
        xT_pool = pool("xT", 3)
        xn_pool = pool("xn", 3)
        Dt_pool = pool("Dt", 3)
        slots_pool = pool("slots", 1)
        ht_pool = pool("ht", 1)
        dd_pool = pool("dd", 2)
        diag_pool = pool("diag", 1)
        out_pool = pool("out", 3)

        # PSUM: 8 banks of 512 f32. ps_small + ps_q = 4 rotating 1-bank
        # tiles for short-lived accumulators (phases 1/2/4 pipeline two
        # groups deep per engine). accA (4 banks) holds the first four
        # slots^T groups in phase 2 and the 4x512 y groups in phase 3;
        # the d=4,5 slots groups live in two ps_q tiles. A start=True
        # matmul clears has_written for its WHOLE bank, so every
        # concurrently-open accumulation group owns a full bank.
        ps_small = pool("ps_small", 2, "PSUM")
        ps_q = pool("ps_q", 2, "PSUM")
        ps_accA = pool("ps_accA", 1, "PSUM")

        phiT_s = singles.tile([128, Dc, SL], F8)
        nc.sync.dma_start(phiT_s[:], phiT[:])
        # Weight DMAs go on the Activation HWDGE queue (emitted after batch
        # 0's phase 1): the Sync queue's in-order issue then only carries
        # the latency-critical xt/xn prefetches, and the 9.4MB of weights
        # don't compete with phase 1's xt tiles at kernel start either.
        w1s = singles.tile([128, Dc, H], BF16)
        w2s = singles.tile([128, Hc, D], BF16)
        b1_s = singles.tile([128, Hc], F32)
        ident = singles.tile([128, 128], F32)
        make_identity(nc, ident[:])
        ebias = singles.tile([128, 1], F32)
        nc.vector.memset(ebias[:], EXP_BIAS)
        y_aug = singles.tile([128, SLc, ODP], F8)
        nc.vector.memset(y_aug[:, :, D : D + 1], SONE)
        nc.vector.memset(y_aug[:, :, D + 1 : ODP], 0.0)

        for b in range(B):
            # ---- phase 1: logits + exp -> E^T (fp8, scaled 2^-4) ----
            eT = eT_pool.tile([128, SLc, N], F8)
            ddp = dd_pool.tile([128, SLc, NT], F32)
            for t in range(NT):
                xt = xT_pool.tile([128, Dc, 512], F8)
                nc.sync.dma_start(xt[:], xT[b, t])
                for s in range(SLc):
                    ps = ps_small.tile([128, 512], F32, tag="pss", name="psL")
                    for dp in range(Dc // 2):
                        nc.tensor.matmul(
                            ps[:],
                            phiT_s[:, 2 * dp : 2 * dp + 2, ts(s, 128)],
                            xt[:, 2 * dp : 2 * dp + 2, :],
                            start=(dp == 0),
                            stop=(dp == Dc // 2 - 1),
                            perf_mode=DR,
                        )
                    nc.scalar.activation(
                        eT[:, s, ts(t, 512)],
                        ps[:],
                        AF.Exp,
                        bias=ebias[:],
                        scale=1.0 / (SX * SPHI),
                        accum_out=ddp[:, s, t : t + 1],
                    )
            if b == 0:
                nc.scalar.dma_start(w1s[:], w1b[:])
                nc.scalar.dma_start(w2s[:], w2b[:])
                nc.scalar.dma_start(b1_s[:], b1[:])
            # ---- dispatch denominators and scaled diagonal blocks ----
            rdd = dd_pool.tile([128, SLc], F32, tag="rdd", name="rdd")
            nc.vector.reduce_sum(rdd[:], ddp[:], axis=mybir.AxisListType.X)
            nc.vector.reciprocal(rdd[:], rdd[:])
            diag = diag_pool.tile([128, SLc, 128], BF16)
            for s in range(SLc):
                nc.vector.tensor_scalar_mul(
                    diag[:, s, :], ident[:], rdd[:, s : s + 1]
                )
            # ---- phase 2: dispatch transpose+normalize, slots^T matmul ----
            # Software-pipelined: norm(v+1) is emitted before slots(v) so
            # the PE runs the next normalize matmuls while the DVE drains
            # psDt(v) into Dt(v).
            accA = ps_accA.tile([128, 2048], F32, tag="accA", name="accA")
            accB4 = ps_q.tile([128, 512], F32, tag="psq", name="accB4")
            accB5 = ps_q.tile([128, 512], F32, tag="psq", name="accB5")
            slot_dst = lambda d: (
                accA[:, d * 512 : (d + 1) * 512]
                if d < 4
                else (accB4 if d == 4 else accB5)[:, :]
            )

            def emit_norm(v):
                psDt = ps_small.tile([128, 512], F32, tag="pss", name="psD")
                for s in range(SLc):
                    nc.tensor.matmul(
                        psDt[:, ts(s, 128)],
                        eT[:, s, ts(v, 128)],
                        diag[:, s, :],
                        start=True,
                        stop=True,
                    )
                Dt = Dt_pool.tile([128, SL], BF16)
                nc.vector.tensor_copy(Dt[:], psDt[:])
                return Dt

            Dt_cur = emit_norm(0)
            for v in range(NV):
                Dt_nxt = emit_norm(v + 1) if v + 1 < NV else None
                xn = xn_pool.tile([128, D], BF16)
                nc.sync.dma_start(xn[:], xnb[b, ts(v, 128), :])
                for d in range(Dc):
                    nc.tensor.matmul(
                        slot_dst(d),
                        xn[:, ts(d, 128)],
                        Dt_cur[:],
                        start=(v == 0),
                        stop=(v == NV - 1),
                    )
                Dt_cur = Dt_nxt
            slotsT = slots_pool.tile([128, Dc, SL], BF16)
            for d in range(Dc):
                src = slot_dst(d)
                if d % 2 == 0:
                    nc.vector.tensor_copy(slotsT[:, d, :], src)
                else:
                    nc.scalar.copy(slotsT[:, d, :], src)
            # ---- phase 3: expert MLP; y accumulated directly in [s, d] ----
            # The first 512 d-columns accumulate in yA (one full PSUM bank
            # per slot chunk — a start=True matmul clears has_written for
            # its WHOLE bank, so concurrently-open groups must not share a
            # bank). The last 256 d-columns are done per slot chunk after
            # the h-loop, each group alone in a rotating ps_small bank.
            yA = ps_accA.tile([128, 2048], F32, tag="accA", name="yA")
            ht_all = ht_pool.tile([128, Hc, SL], BF16)

            def emit_y512(h):
                for sc in range(SLc):
                    nc.tensor.matmul(
                        yA[:, sc * 512 : (sc + 1) * 512],
                        ht_all[:, h, ts(sc, 128)],
                        w2s[:, h, 0:512],
                        start=(h == 0),
                        stop=(h == Hc - 1),
                    )

            # Software-pipelined: y matmuls for h-1 are emitted after the
            # psh matmuls for h, so the PE runs them while ACT computes
            # gelu(h) instead of stalling on it.
            for h in range(Hc):
                psh = ps_small.tile([128, 512], F32, tag="pss", name="psH")
                for d in range(Dc):
                    nc.tensor.matmul(
                        psh[:],
                        w1s[:, d, ts(h, 128)],
                        slotsT[:, d, :],
                        start=(d == 0),
                        stop=(d == Dc - 1),
                    )
                nc.scalar.activation(
                    ht_all[:, h, :], psh[:], act_fn, bias=b1_s[:, h : h + 1]
                )
                if h > 0:
                    emit_y512(h - 1)
            emit_y512(Hc - 1)
            for sc in range(SLc):
                nc.vector.tensor_scalar_mul(
                    y_aug[:, sc, 0:512], yA[:, sc * 512 : (sc + 1) * 512], SY
                )
            for sc in range(SLc):
                pool_ = ps_small if sc % 2 == 0 else ps_q
                tag_ = "pss" if sc % 2 == 0 else "psq"
                psB = pool_.tile([128, 512], F32, tag=tag_, name="psB")
                for h in range(Hc):
                    nc.tensor.matmul(
                        psB[:, :256],
                        ht_all[:, h, ts(sc, 128)],
                        w2s[:, h, 512:768],
                        start=(h == 0),
                        stop=(h == Hc - 1),
                    )
                nc.scalar.activation(
                    y_aug[:, sc, 512:768], psB[:, :256], AF.Copy, scale=SY
                )
            # ---- phase 4: combine partials + local denominator ----
            for v in range(NV):
                ot = out_pool.tile([128, OD], BF16)
                for gi, (off, sz) in enumerate(((0, 512), (512, ODP - 512))):
                    if gi == 0:
                        pso = ps_small.tile([128, 512], F32, tag="pss", name="psO")
                    else:
                        pso = ps_q.tile([128, 512], F32, tag="psq", name="psO")
                    for sp in range(SLc // 2):
                        nc.tensor.matmul(
                            pso[:, :sz],
                            eT[:, 2 * sp : 2 * sp + 2, ts(v, 128)],
                            y_aug[:, 2 * sp : 2 * sp + 2, off : off + sz],
                            start=(sp == 0),
                            stop=(sp == SLc // 2 - 1),
                            perf_mode=DR,
                        )
                    osz = min(sz, OD - off)
                    if off == 0:
                        nc.scalar.copy(ot[:, off : off + osz], pso[:, :osz])
                    else:
                        nc.vector.tensor_copy(ot[:, off : off + osz], pso[:, :osz])
                nc.sync.dma_start(outp[b, ts(v, 128), :], ot[:])

    return nc


def _make_core_inputs(x, phi, w1, b1, w2, n_cores=N_CORES):
    B, N, Dd = x.shape
    E, S, _ = phi.shape
    H = w1.shape[2]
    halves = n_cores // E
    SL = S // halves
    Dc, Hc = Dd // 128, H // 128
    NT = N // 512
    xs = SX * x
    assert np.max(np.abs(xs)) < 200.0
    # [b, t, p, k, ni] with token n = t*512 + ni, d = k*128 + p
    xT_full = np.ascontiguousarray(
        xs.reshape(B, NT, 512, Dc, 128).transpose(0, 1, 4, 3, 2)
    ).astype(F8NP)
    xnb = np.ascontiguousarray(x).astype(BF)
    in_maps = []
    for c in range(n_cores):
        e, hh = c // halves, c % halves
        phi_loc = SPHI * phi[e, hh * SL : (hh + 1) * SL, :]
        assert np.max(np.abs(phi_loc)) < 200.0
        phiT = np.ascontiguousarray(
            phi_loc.T.reshape(Dc, 128, SL).transpose(1, 0, 2)
        ).astype(F8NP)
        in_maps.append(
            {
                "xT": xT_full,
                "xnb": xnb,
                "phiT": phiT,
                "w1b": np.ascontiguousarray(
                    w1[e].reshape(Dc, 128, H).transpose(1, 0, 2)
                ).astype(BF),
                "w2b": np.ascontiguousarray(
                    w2[e].reshape(Hc, 128, Dd).transpose(1, 0, 2)
                ).astype(BF),
                "b1": np.ascontiguousarray(b1[e].reshape(Hc, 128).T),
            }
        )
    return in_maps


def _combine_core_outputs(outs, b2, n_cores=N_CORES):
    E, D = b2.shape
    halves = n_cores // E
    num = np.zeros(outs[0]["outp"][..., :D].shape, dtype=np.float64)
    den = np.zeros(outs[0]["outp"][..., D].shape, dtype=np.float64)
    num_scale = 1.0 / (2.0**SE_LOG2 * SY)
    gdl_scale = 1.0 / (2.0**SE_LOG2 * SONE)
    for c, r in enumerate(outs):
        e = c // halves
        o = r["outp"].astype(np.float64)
        gdl = o[..., D] * gdl_scale
        num += o[..., :D] * num_scale
        num += gdl[..., None] * b2[e].astype(np.float64)[None, None, :]
        den += gdl
    return (num / den[..., None]).astype(np.float32)


def _run(x, phi, w1, b1, w2, b2, trace=False, tmpdir=None):
    x = np.asarray(x, dtype=np.float32)
    phi = np.asarray(phi, dtype=np.float32)
    w1 = np.asarray(w1, dtype=np.float32)
    b1 = np.asarray(b1, dtype=np.float32)
    w2 = np.asarray(w2, dtype=np.float32)
    b2 = np.asarray(b2, dtype=np.float32)

    B, N, D = x.shape
    E, S, _ = phi.shape
    H = w1.shape[2]
    SL = S // (N_CORES // E)

    nc = bass.Bass(
        "TRN2", target_bir_lowering=False, debug=False, num_devices=N_CORES
    )
    _emit_moe_kernel(nc, B, N, D, SL, H)
    _split_excess_waits(nc)

    in_maps = _make_core_inputs(x, phi, w1, b1, w2)
    res = run_bass_kernel_spmd(
        nc, in_maps, core_ids=list(range(N_CORES)), trace=trace, tmpdir=tmpdir
    )
    return _combine_core_outputs(res.results, b2), res


def kernel(x, phi, w1, b1, w2, b2):
    return _run(x, phi, w1, b1, w2, b2)[0]


# revision 42
# speedup vs baseline: 1.1433x; 1.1433x over previous
"""Trainium2 Bass kernel for nn_MixtureExpertsMlp (MoE soft routing).

Contract: kernel(**inputs) takes the FULL unsharded inputs
(x [4,4096,768], phi [4,1024,768], w1 [4,768,3072], b1 [4,3072],
w2 [4,3072,768], b2 [4,768]) and returns the FULL output [4,4096,768].

Sharding (expert+slot parallel over 8 NeuronCores): core c owns expert
e = c // 2 and slot half h = c % 2, i.e. SL = 512 of that expert's 1024
routing slots. Every core sees all tokens. Per core and per batch b:

  L^T[s, n]    = sum_d phi[s, d] x[b, n, d]        (slots on partitions)
  E^T          = exp(L^T)          (softmax max-subtraction skipped: the
                                    logits are ~N(0,1), well within fp32)
  ddenom[s]    = sum_n E^T[s, n]                    (via ACT accum_out)
  D[n, s]      = E^T[s, n] / ddenom[s]    (dispatch; transposed+normalized
                                    per 128-slot block in one matmul
                                    against diag(1/ddenom) blocks)
  slots^T[d,s] = sum_n x[b, n, d] D[n, s]
  h^T[h', s]   = gelu_tanh(sum_d w1[d, h'] slots^T[d, s] + b1[h'])
  y[s, d]      = sum_h h^T[h, s]^T w2[h, d]   (directly in [s, d] layout:
                                    stationary = h^T chunk, moving = w2 row)
  outp[n, :D]  = sum_s E^T[s, n] y[s, :]      (unnormalized combine)
  outp[n, D]   = sum_s E^T[s, n]              (ones column appended to y)

Host-side unshard: the combine softmax normalizer is global over all
E*S slots, so out = (sum_c num_c + sum_c gdl_c * b2[e(c)]) / sum_c gdl_c
where num_c = outp_c[..., :D] and gdl_c = outp_c[..., D]. This also
folds in b2 exactly (per-expert combine mass times b2[e]).

Precision: logits matmul in float32r (exp amplifies logit error into
routing-weight relative error). Everything downstream (dispatch
weights, slots, MLP, combine) in bf16 — ~1e-3 relative noise against a
2e-2 gate. w1/w2 live in SBUF in bf16 for the whole kernel (loaded
once, not per batch); outputs are written bf16 and combined on host in
float64.
"""

import numpy as np
from contextlib import ExitStack

import ml_dtypes

import concourse.bass as bass
import concourse.tile as tile
from concourse import mybir
from concourse.bass import ts
from concourse.masks import make_identity
from concourse.bass_utils import run_bass_kernel_spmd

F32 = mybir.dt.float32
F32R = mybir.dt.float32r
BF16 = mybir.dt.bfloat16
F8 = mybir.dt.float8e4
AF = mybir.ActivationFunctionType
DR = mybir.MatmulPerfMode.DoubleRow
BF = ml_dtypes.bfloat16
F8NP = ml_dtypes.float8_e4m3  # TRN e4m3: max +-240, inf beyond — matches HW

N_CORES = 8
# fp8 logits scales: logits_psum = (SX*x) @ (SPHI*phi); exp() applies 1/(SX*SPHI).
SX = 16.0
SPHI = 512.0
# fp8 combine scales: eT stored as 2^-4*exp(L) (via the exp bias), y stored as
# SY*y, ones column stored as SONE. Host unscales consistently.
SE_LOG2 = -4
SY = 256.0
SONE = 0.125
import math

EXP_BIAS = SE_LOG2 * math.log(2.0)


# --------------------------------------------------------------------------
# Post-pass: the walrus build in this container enforces the ISA cap of one
# sync-wait per instruction (two for EventSemaphore); Tile's final drain can
# carry more. Hoist excess waits onto fresh same-engine NOPs.
# --------------------------------------------------------------------------
def _split_excess_waits(nc):
    caps = {"InstEventSemaphore": 2}
    n_new = 0
    for f in nc.m.functions:
        for bb in f.blocks:
            i = 0
            insts = bb.instructions
            while i < len(insts):
                ins = insts[i]
                si = ins.sync_info
                cap = caps.get(type(ins).__name__, 1)
                if si is not None and len(si.on_wait) > cap:
                    waits = list(si.on_wait)
                    keep, hoist = waits[-cap:], waits[:-cap]
                    new_nops = []
                    for w in hoist:
                        nop = mybir.InstNoOp(
                            name=nc.get_next_instruction_name(),
                            engine=ins.engine,
                            ins=[],
                            outs=[],
                            sync_info=mybir.SyncInfo(on_wait=[w], on_update=[]),
                        )
                        nc.register_instruction(nop)
                        new_nops.append(nop)
                    ins.sync_info = mybir.SyncInfo(
                        on_wait=keep, on_update=list(si.on_update)
                    )
                    insts[i:i] = new_nops
                    i += len(new_nops)
                    n_new += len(new_nops)
                i += 1
    return n_new


def _emit_moe_kernel(nc, B, N, D, SL, H, act_fn=AF.Gelu_apprx_tanh):
    assert N % 512 == 0 and D % 128 == 0 and SL % 128 == 0 and H % 128 == 0
    Dc, SLc, Hc = D // 128, SL // 128, H // 128
    NT, NV = N // 512, N // 128
    OD = D + 2  # output: D columns + ones column (combine denom) + pad
    ODP = D + 16  # y_aug width: DoubleRow needs the pair step % 16 == 0

    xT = nc.dram_tensor("xT", [B, Dc, 128, N], F8, kind="ExternalInput").ap()
    xnb = nc.dram_tensor("xnb", [B, N, D], BF16, kind="ExternalInput").ap()
    phiT = nc.dram_tensor("phiT", [Dc, 128, SL], F8, kind="ExternalInput").ap()
    w1b = nc.dram_tensor("w1b", [Dc, 128, H], BF16, kind="ExternalInput").ap()
    w2b = nc.dram_tensor("w2b", [Hc, 128, D], BF16, kind="ExternalInput").ap()
    b1 = nc.dram_tensor("b1", [Hc, 128], F32, kind="ExternalInput").ap()
    outp = nc.dram_tensor("outp", [B, N, OD], BF16, kind="ExternalOutput").ap()

    with tile.TileContext(nc) as tc, ExitStack() as ctx:
        pool = lambda name, bufs, space="SBUF": ctx.enter_context(
            tc.tile_pool(name=name, bufs=bufs, space=space)
        )
        singles = pool("singles", 1)
        eT_pool = pool("eT", 1)
        xT_pool = pool("xT", 3)
        xn_pool = pool("xn", 3)
        Dt_pool = pool("Dt", 3)
        slots_pool = pool("slots", 1)
        ht_pool = pool("ht", 1)
        dd_pool = pool("dd", 2)
        diag_pool = pool("diag", 1)
        out_pool = pool("out", 3)

        # PSUM: 8 banks of 512 f32. ps_small + ps_q = 4 rotating 1-bank
        # tiles for short-lived accumulators (phases 1/2/4 pipeline two
        # groups deep per engine). accA (4 banks) holds the first four
        # slots^T groups in phase 2 and the 4x512 y groups in phase 3;
        # the d=4,5 slots groups live in two ps_q tiles. A start=True
        # matmul clears has_written for its WHOLE bank, so every
        # concurrently-open accumulation group owns a full bank.
        ps_small = pool("ps_small", 2, "PSUM")
        ps_q = pool("ps_q", 2, "PSUM")
        ps_accA = pool("ps_accA", 1, "PSUM")

        phiT_s = singles.tile([128, Dc, SL], F8)
        nc.sync.dma_start(phiT_s[:], phiT.rearrange("k p m -> p k m"))
        # Weight DMAs go on the Activation HWDGE queue (emitted after batch
        # 0's phase 1): the Sync queue's in-order issue then only carries
        # the latency-critical xt/xn prefetches, and the 9.4MB of weights
        # don't compete with phase 1's xt tiles at kernel start either.
        w1s = singles.tile([128, Dc, H], BF16)
        w2s = singles.tile([128, Hc, D], BF16)
        b1_s = singles.tile([128, Hc], F32)
        ident = singles.tile([128, 128], F32)
        make_identity(nc, ident[:])
        ebias = singles.tile([128, 1], F32)
        nc.vector.memset(ebias[:], EXP_BIAS)
        y_aug = singles.tile([128, SLc, ODP], F8)
        nc.vector.memset(y_aug[:, :, D : D + 1], SONE)
        nc.vector.memset(y_aug[:, :, D + 1 : ODP], 0.0)

        for b in range(B):
            # ---- phase 1: logits + exp -> E^T (fp8, scaled 2^-4) ----
            eT = eT_pool.tile([128, SLc, N], F8)
            ddp = dd_pool.tile([128, SLc, NT], F32)
            for t in range(NT):
                xt = xT_pool.tile([128, Dc, 512], F8)
                nc.sync.dma_start(
                    xt[:], xT[b, :, :, ts(t, 512)].rearrange("k p n -> p k n")
                )
                for s in range(SLc):
                    ps = ps_small.tile([128, 512], F32, tag="pss", name="psL")
                    for dp in range(Dc // 2):
                        nc.tensor.matmul(
                            ps[:],
                            phiT_s[:, 2 * dp : 2 * dp + 2, ts(s, 128)],
                            xt[:, 2 * dp : 2 * dp + 2, :],
                            start=(dp == 0),
                            stop=(dp == Dc // 2 - 1),
                            perf_mode=DR,
                        )
                    nc.scalar.activation(
                        eT[:, s, ts(t, 512)],
                        ps[:],
                        AF.Exp,
                        bias=ebias[:],
                        scale=1.0 / (SX * SPHI),
                        accum_out=ddp[:, s, t : t + 1],
                    )
            if b == 0:
                nc.scalar.dma_start(w1s[:], w1b.rearrange("k p m -> p k m"))
                nc.scalar.dma_start(w2s[:], w2b.rearrange("k p m -> p k m"))
                nc.scalar.dma_start(b1_s[:], b1.rearrange("o p -> p o"))
            # ---- dispatch denominators and scaled diagonal blocks ----
            rdd = dd_pool.tile([128, SLc], F32, tag="rdd", name="rdd")
            nc.vector.reduce_sum(rdd[:], ddp[:], axis=mybir.AxisListType.X)
            nc.vector.reciprocal(rdd[:], rdd[:])
            diag = diag_pool.tile([128, SLc, 128], BF16)
            for s in range(SLc):
                nc.vector.tensor_scalar_mul(
                    diag[:, s, :], ident[:], rdd[:, s : s + 1]
                )
            # ---- phase 2: dispatch transpose+normalize, slots^T matmul ----
            # Software-pipelined: norm(v+1) is emitted before slots(v) so
            # the PE runs the next normalize matmuls while the DVE drains
            # psDt(v) into Dt(v).
            accA = ps_accA.tile([128, 2048], F32, tag="accA", name="accA")
            accB4 = ps_q.tile([128, 512], F32, tag="psq", name="accB4")
            accB5 = ps_q.tile([128, 512], F32, tag="psq", name="accB5")
            slot_dst = lambda d: (
                accA[:, d * 512 : (d + 1) * 512]
                if d < 4
                else (accB4 if d == 4 else accB5)[:, :]
            )

            def emit_norm(v):
                psDt = ps_small.tile([128, 512], F32, tag="pss", name="psD")
                for s in range(SLc):
                    nc.tensor.matmul(
                        psDt[:, ts(s, 128)],
                        eT[:, s, ts(v, 128)],
                        diag[:, s, :],
                        start=True,
                        stop=True,
                    )
                Dt = Dt_pool.tile([128, SL], BF16)
                nc.vector.tensor_copy(Dt[:], psDt[:])
                return Dt

            Dt_cur = emit_norm(0)
            for v in range(NV):
                Dt_nxt = emit_norm(v + 1) if v + 1 < NV else None
                xn = xn_pool.tile([128, D], BF16)
                nc.sync.dma_start(xn[:], xnb[b, ts(v, 128), :])
                for d in range(Dc):
                    nc.tensor.matmul(
                        slot_dst(d),
                        xn[:, ts(d, 128)],
                        Dt_cur[:],
                        start=(v == 0),
                        stop=(v == NV - 1),
                    )
                Dt_cur = Dt_nxt
            slotsT = slots_pool.tile([128, Dc, SL], BF16)
            for d in range(Dc):
                src = slot_dst(d)
                if d % 2 == 0:
                    nc.vector.tensor_copy(slotsT[:, d, :], src)
                else:
                    nc.scalar.copy(slotsT[:, d, :], src)
            # ---- phase 3: expert MLP; y accumulated directly in [s, d] ----
            # The first 512 d-columns accumulate in yA (one full PSUM bank
            # per slot chunk — a start=True matmul clears has_written for
            # its WHOLE bank, so concurrently-open groups must not share a
            # bank). The last 256 d-columns are done per slot chunk after
            # the h-loop, each group alone in a rotating ps_small bank.
            yA = ps_accA.tile([128, 2048], F32, tag="accA", name="yA")
            ht_all = ht_pool.tile([128, Hc, SL], BF16)

            def emit_y512(h):
                for sc in range(SLc):
                    nc.tensor.matmul(
                        yA[:, sc * 512 : (sc + 1) * 512],
                        ht_all[:, h, ts(sc, 128)],
                        w2s[:, h, 0:512],
                        start=(h == 0),
                        stop=(h == Hc - 1),
                    )

            # Software-pipelined: y matmuls for h-1 are emitted after the
            # psh matmuls for h, so the PE runs them while ACT computes
            # gelu(h) instead of stalling on it.
            for h in range(Hc):
                psh = ps_small.tile([128, 512], F32, tag="pss", name="psH")
                for d in range(Dc):
                    nc.tensor.matmul(
                        psh[:],
                        w1s[:, d, ts(h, 128)],
                        slotsT[:, d, :],
                        start=(d == 0),
                        stop=(d == Dc - 1),
                    )
                nc.scalar.activation(
                    ht_all[:, h, :], psh[:], act_fn, bias=b1_s[:, h : h + 1]
                )
                if h > 0:
                    emit_y512(h - 1)
            emit_y512(Hc - 1)
            for sc in range(SLc):
                nc.vector.tensor_scalar_mul(
                    y_aug[:, sc, 0:512], yA[:, sc * 512 : (sc + 1) * 512], SY
                )
            for sc in range(SLc):
                pool_ = ps_small if sc % 2 == 0 else ps_q
                tag_ = "pss" if sc % 2 == 0 else "psq"
                psB = pool_.tile([128, 512], F32, tag=tag_, name="psB")
                for h in range(Hc):
                    nc.tensor.matmul(
                        psB[:, :256],
                        ht_all[:, h, ts(sc, 128)],
                        w2s[:, h, 512:768],
                        start=(h == 0),
                        stop=(h == Hc - 1),
                    )
                nc.scalar.activation(
                    y_aug[:, sc, 512:768], psB[:, :256], AF.Copy, scale=SY
                )
            # ---- phase 4: combine partials + local denominator ----
            for v in range(NV):
                ot = out_pool.tile([128, OD], BF16)
                for gi, (off, sz) in enumerate(((0, 512), (512, ODP - 512))):
                    if gi == 0:
                        pso = ps_small.tile([128, 512], F32, tag="pss", name="psO")
                    else:
                        pso = ps_q.tile([128, 512], F32, tag="psq", name="psO")
                    for sp in range(SLc // 2):
                        nc.tensor.matmul(
                            pso[:, :sz],
                            eT[:, 2 * sp : 2 * sp + 2, ts(v, 128)],
                            y_aug[:, 2 * sp : 2 * sp + 2, off : off + sz],
                            start=(sp == 0),
                            stop=(sp == SLc // 2 - 1),
                            perf_mode=DR,
                        )
                    osz = min(sz, OD - off)
                    if off == 0:
                        nc.scalar.copy(ot[:, off : off + osz], pso[:, :osz])
                    else:
                        nc.vector.tensor_copy(ot[:, off : off + osz], pso[:, :osz])
                nc.sync.dma_start(outp[b, ts(v, 128), :], ot[:])

    return nc


def _make_core_inputs(x, phi, w1, b1, w2, n_cores=N_CORES):
    B, N, Dd = x.shape
    E, S, _ = phi.shape
    H = w1.shape[2]
    halves = n_cores // E
    SL = S // halves
    Dc, Hc = Dd // 128, H // 128
    xs = SX * x
    assert np.max(np.abs(xs)) < 200.0
    xT_full = np.ascontiguousarray(xs.transpose(0, 2, 1)).reshape(
        B, Dc, 128, N
    ).astype(F8NP)
    xnb = np.ascontiguousarray(x).astype(BF)
    in_maps = []
    for c in range(n_cores):
        e, hh = c // halves, c % halves
        phi_loc = SPHI * phi[e, hh * SL : (hh + 1) * SL, :]
        assert np.max(np.abs(phi_loc)) < 200.0
        phiT = np.ascontiguousarray(phi_loc.T).reshape(Dc, 128, SL).astype(F8NP)
        in_maps.append(
            {
                "xT": xT_full,
                "xnb": xnb,
                "phiT": phiT,
                "w1b": np.ascontiguousarray(w1[e]).reshape(Dc, 128, H).astype(BF),
                "w2b": np.ascontiguousarray(w2[e]).reshape(Hc, 128, Dd).astype(BF),
                "b1": np.ascontiguousarray(b1[e]).reshape(Hc, 128),
            }
        )
    return in_maps


def _combine_core_outputs(outs, b2, n_cores=N_CORES):
    E, D = b2.shape
    halves = n_cores // E
    num = np.zeros(outs[0]["outp"][..., :D].shape, dtype=np.float64)
    den = np.zeros(outs[0]["outp"][..., D].shape, dtype=np.float64)
    num_scale = 1.0 / (2.0**SE_LOG2 * SY)
    gdl_scale = 1.0 / (2.0**SE_LOG2 * SONE)
    for c, r in enumerate(outs):
        e = c // halves
        o = r["outp"].astype(np.float64)
        gdl = o[..., D] * gdl_scale
        num += o[..., :D] * num_scale
        num += gdl[..., None] * b2[e].astype(np.float64)[None, None, :]
        den += gdl
    return (num / den[..., None]).astype(np.float32)


def _run(x, phi, w1, b1, w2, b2, trace=False, tmpdir=None):
    x = np.asarray(x, dtype=np.float32)
    phi = np.asarray(phi, dtype=np.float32)
    w1 = np.asarray(w1, dtype=np.float32)
    b1 = np.asarray(b1, dtype=np.float32)
    w2 = np.asarray(w2, dtype=np.float32)
    b2 = np.asarray(b2, dtype=np.float32)

    B, N, D = x.shape
    E, S, _ = phi.shape
    H = w1.shape[2]
    SL = S // (N_CORES // E)

    nc = bass.Bass(
        "TRN2", target_bir_lowering=False, debug=False, num_devices=N_CORES
    )
    _emit_moe_kernel(nc, B, N, D, SL, H)
    _split_excess_waits(nc)

    in_maps = _make_core_inputs(x, phi, w1, b1, w2)
    res = run_bass_kernel_spmd(
        nc, in_maps, core_ids=list(range(N_CORES)), trace=trace, tmpdir=tmpdir
    )
    return _combine_core_outputs(res.results, b2), res


def kernel(x, phi, w1, b1, w2, b2):
    return _run(x, phi, w1, b1, w2, b2)[0]


# revision 43
# speedup vs baseline: 1.1910x; 1.0418x over previous
"""Trainium2 Bass kernel for nn_MixtureExpertsMlp (MoE soft routing).

Contract: kernel(**inputs) takes the FULL unsharded inputs
(x [4,4096,768], phi [4,1024,768], w1 [4,768,3072], b1 [4,3072],
w2 [4,3072,768], b2 [4,768]) and returns the FULL output [4,4096,768].

Sharding (expert+slot parallel over 8 NeuronCores): core c owns expert
e = c // 2 and slot half h = c % 2, i.e. SL = 512 of that expert's 1024
routing slots. Every core sees all tokens. Per core and per batch b:

  L^T[s, n]    = sum_d phi[s, d] x[b, n, d]        (slots on partitions)
  E^T          = exp(L^T)          (softmax max-subtraction skipped: the
                                    logits are ~N(0,1), well within fp32)
  ddenom[s]    = sum_n E^T[s, n]                    (via ACT accum_out)
  D[n, s]      = E^T[s, n] / ddenom[s]    (dispatch; transposed+normalized
                                    per 128-slot block in one matmul
                                    against diag(1/ddenom) blocks)
  slots^T[d,s] = sum_n x[b, n, d] D[n, s]
  h^T[h', s]   = gelu_tanh(sum_d w1[d, h'] slots^T[d, s] + b1[h'])
  y[s, d]      = sum_h h^T[h, s]^T w2[h, d]   (directly in [s, d] layout:
                                    stationary = h^T chunk, moving = w2 row)
  outp[n, :D]  = sum_s E^T[s, n] y[s, :]      (unnormalized combine)
  outp[n, D]   = sum_s E^T[s, n]              (ones column appended to y)

Host-side unshard: the combine softmax normalizer is global over all
E*S slots, so out = (sum_c num_c + sum_c gdl_c * b2[e(c)]) / sum_c gdl_c
where num_c = outp_c[..., :D] and gdl_c = outp_c[..., D]. This also
folds in b2 exactly (per-expert combine mass times b2[e]).

Precision: logits matmul in float32r (exp amplifies logit error into
routing-weight relative error). Everything downstream (dispatch
weights, slots, MLP, combine) in bf16 — ~1e-3 relative noise against a
2e-2 gate. w1/w2 live in SBUF in bf16 for the whole kernel (loaded
once, not per batch); outputs are written bf16 and combined on host in
float64.
"""

import numpy as np
from contextlib import ExitStack

import ml_dtypes

import concourse.bass as bass
import concourse.tile as tile
from concourse import mybir
from concourse.bass import ts
from concourse.masks import make_identity
from concourse.bass_utils import run_bass_kernel_spmd

F32 = mybir.dt.float32
F32R = mybir.dt.float32r
BF16 = mybir.dt.bfloat16
F8 = mybir.dt.float8e4
AF = mybir.ActivationFunctionType
DR = mybir.MatmulPerfMode.DoubleRow
BF = ml_dtypes.bfloat16
F8NP = ml_dtypes.float8_e4m3  # TRN e4m3: max +-240, inf beyond — matches HW

N_CORES = 8
# fp8 logits scales: logits_psum = (SX*x) @ (SPHI*phi); exp() applies 1/(SX*SPHI).
SX = 16.0
SPHI = 512.0
# fp8 combine scales: eT stored as 2^-4*exp(L) (via the exp bias), y stored as
# SY*y, ones column stored as SONE. Host unscales consistently.
SE_LOG2 = -4
SY = 256.0
SONE = 0.125
import math

EXP_BIAS = SE_LOG2 * math.log(2.0)


# --------------------------------------------------------------------------
# Post-pass: the walrus build in this container enforces the ISA cap of one
# sync-wait per instruction (two for EventSemaphore); Tile's final drain can
# carry more. Hoist excess waits onto fresh same-engine NOPs.
# --------------------------------------------------------------------------
def _split_excess_waits(nc):
    caps = {"InstEventSemaphore": 2}
    n_new = 0
    for f in nc.m.functions:
        for bb in f.blocks:
            i = 0
            insts = bb.instructions
            while i < len(insts):
                ins = insts[i]
                si = ins.sync_info
                cap = caps.get(type(ins).__name__, 1)
                if si is not None and len(si.on_wait) > cap:
                    waits = list(si.on_wait)
                    keep, hoist = waits[-cap:], waits[:-cap]
                    new_nops = []
                    for w in hoist:
                        nop = mybir.InstNoOp(
                            name=nc.get_next_instruction_name(),
                            engine=ins.engine,
                            ins=[],
                            outs=[],
                            sync_info=mybir.SyncInfo(on_wait=[w], on_update=[]),
                        )
                        nc.register_instruction(nop)
                        new_nops.append(nop)
                    ins.sync_info = mybir.SyncInfo(
                        on_wait=keep, on_update=list(si.on_update)
                    )
                    insts[i:i] = new_nops
                    i += len(new_nops)
                    n_new += len(new_nops)
                i += 1
    return n_new


def _emit_moe_kernel(nc, B, N, D, SL, H, act_fn=AF.Gelu_apprx_tanh):
    assert N % 512 == 0 and D % 128 == 0 and SL % 128 == 0 and H % 128 == 0
    Dc, SLc, Hc = D // 128, SL // 128, H // 128
    NT, NV = N // 512, N // 128
    OD = D + 2  # output: D columns + ones column (combine denom) + pad
    ODP = D + 16  # y_aug width: DoubleRow needs the pair step % 16 == 0

    xT = nc.dram_tensor("xT", [B, Dc, 128, N], F8, kind="ExternalInput").ap()
    xnb = nc.dram_tensor("xnb", [B, N, D], BF16, kind="ExternalInput").ap()
    phiT = nc.dram_tensor("phiT", [Dc, 128, SL], F8, kind="ExternalInput").ap()
    w1b = nc.dram_tensor("w1b", [Dc, 128, H], BF16, kind="ExternalInput").ap()
    w2b = nc.dram_tensor("w2b", [Hc, 128, D], BF16, kind="ExternalInput").ap()
    b1 = nc.dram_tensor("b1", [Hc, 128], F32, kind="ExternalInput").ap()
    outp = nc.dram_tensor("outp", [B, N, OD], BF16, kind="ExternalOutput").ap()

    with tile.TileContext(nc) as tc, ExitStack() as ctx:
        pool = lambda name, bufs, space="SBUF": ctx.enter_context(
            tc.tile_pool(name=name, bufs=bufs, space=space)
        )
        singles = pool("singles", 1)
        eT_pool = pool("eT", 1)
        xT_pool = pool("xT", 3)
        xn_pool = pool("xn", 3)
        Dt_pool = pool("Dt", 3)
        slots_pool = pool("slots", 1)
        ht_pool = pool("ht", 1)
        dd_pool = pool("dd", 2)
        diag_pool = pool("diag", 1)
        out_pool = pool("out", 3)

        # PSUM: 8 banks of 512 f32. ps_small + ps_q = 4 rotating 1-bank
        # tiles for short-lived accumulators (phases 1/2/4 pipeline two
        # groups deep per engine). accA (4 banks) holds the first four
        # slots^T groups in phase 2 and the 4x512 y groups in phase 3;
        # the d=4,5 slots groups live in two ps_q tiles. A start=True
        # matmul clears has_written for its WHOLE bank, so every
        # concurrently-open accumulation group owns a full bank.
        ps_small = pool("ps_small", 2, "PSUM")
        ps_q = pool("ps_q", 2, "PSUM")
        ps_accA = pool("ps_accA", 1, "PSUM")

        # phiT issues from the ACT HWDGE queue so it overlaps the first xt
        # prefetches on the Sync queue (the cold DMA path costs ~10us per
        # first-transfer on each queue; don't serialize the two).
        phiT_s = singles.tile([128, Dc, SL], F8)
        nc.scalar.dma_start(phiT_s[:], phiT.rearrange("k p m -> p k m"))
        # Weight DMAs go on the Activation HWDGE queue (emitted after batch
        # 0's phase 1): the Sync queue's in-order issue then only carries
        # the latency-critical xt/xn prefetches, and the 9.4MB of weights
        # don't compete with phase 1's xt tiles at kernel start either.
        w1s = singles.tile([128, Dc, H], BF16)
        w2s = singles.tile([128, Hc, D], BF16)
        b1_s = singles.tile([128, Hc], F32)
        ident = singles.tile([128, 128], F32)
        make_identity(nc, ident[:])
        ebias = singles.tile([128, 1], F32)
        nc.vector.memset(ebias[:], EXP_BIAS)
        y_aug = singles.tile([128, SLc, ODP], F8)
        nc.vector.memset(y_aug[:, :, D : D + 1], SONE)
        nc.vector.memset(y_aug[:, :, D + 1 : ODP], 0.0)

        for b in range(B):
            # ---- phase 1: logits + exp -> E^T (fp8, scaled 2^-4) ----
            eT = eT_pool.tile([128, SLc, N], F8)
            ddp = dd_pool.tile([128, SLc, NT], F32)
            for t in range(NT):
                xt = xT_pool.tile([128, Dc, 512], F8)
                nc.sync.dma_start(
                    xt[:], xT[b, :, :, ts(t, 512)].rearrange("k p n -> p k n")
                )
                for s in range(SLc):
                    ps = ps_small.tile([128, 512], F32, tag="pss", name="psL")
                    for dp in range(Dc // 2):
                        nc.tensor.matmul(
                            ps[:],
                            phiT_s[:, 2 * dp : 2 * dp + 2, ts(s, 128)],
                            xt[:, 2 * dp : 2 * dp + 2, :],
                            start=(dp == 0),
                            stop=(dp == Dc // 2 - 1),
                            perf_mode=DR,
                        )
                    nc.scalar.activation(
                        eT[:, s, ts(t, 512)],
                        ps[:],
                        AF.Exp,
                        bias=ebias[:],
                        scale=1.0 / (SX * SPHI),
                        accum_out=ddp[:, s, t : t + 1],
                    )
            if b == 0:
                nc.scalar.dma_start(w1s[:], w1b.rearrange("k p m -> p k m"))
                nc.scalar.dma_start(w2s[:], w2b.rearrange("k p m -> p k m"))
                nc.scalar.dma_start(b1_s[:], b1.rearrange("o p -> p o"))
            # ---- dispatch denominators and scaled diagonal blocks ----
            rdd = dd_pool.tile([128, SLc], F32, tag="rdd", name="rdd")
            nc.vector.reduce_sum(rdd[:], ddp[:], axis=mybir.AxisListType.X)
            nc.vector.reciprocal(rdd[:], rdd[:])
            diag = diag_pool.tile([128, SLc, 128], BF16)
            for s in range(SLc):
                nc.vector.tensor_scalar_mul(
                    diag[:, s, :], ident[:], rdd[:, s : s + 1]
                )
            # ---- phase 2: dispatch transpose+normalize, slots^T matmul ----
            # Software-pipelined: norm(v+1) is emitted before slots(v) so
            # the PE runs the next normalize matmuls while the DVE drains
            # psDt(v) into Dt(v).
            accA = ps_accA.tile([128, 2048], F32, tag="accA", name="accA")
            accB4 = ps_q.tile([128, 512], F32, tag="psq", name="accB4")
            accB5 = ps_q.tile([128, 512], F32, tag="psq", name="accB5")
            slot_dst = lambda d: (
                accA[:, d * 512 : (d + 1) * 512]
                if d < 4
                else (accB4 if d == 4 else accB5)[:, :]
            )

            def emit_norm(v):
                psDt = ps_small.tile([128, 512], F32, tag="pss", name="psD")
                for s in range(SLc):
                    nc.tensor.matmul(
                        psDt[:, ts(s, 128)],
                        eT[:, s, ts(v, 128)],
                        diag[:, s, :],
                        start=True,
                        stop=True,
                    )
                Dt = Dt_pool.tile([128, SL], BF16)
                nc.vector.tensor_copy(Dt[:], psDt[:])
                return Dt

            Dt_cur = emit_norm(0)
            for v in range(NV):
                Dt_nxt = emit_norm(v + 1) if v + 1 < NV else None
                xn = xn_pool.tile([128, D], BF16)
                nc.sync.dma_start(xn[:], xnb[b, ts(v, 128), :])
                for d in range(Dc):
                    nc.tensor.matmul(
                        slot_dst(d),
                        xn[:, ts(d, 128)],
                        Dt_cur[:],
                        start=(v == 0),
                        stop=(v == NV - 1),
                    )
                Dt_cur = Dt_nxt
            slotsT = slots_pool.tile([128, Dc, SL], BF16)
            for d in range(Dc):
                src = slot_dst(d)
                if d % 2 == 0:
                    nc.vector.tensor_copy(slotsT[:, d, :], src)
                else:
                    nc.scalar.copy(slotsT[:, d, :], src)
            # ---- phase 3: expert MLP; y accumulated directly in [s, d] ----
            # The first 512 d-columns accumulate in yA (one full PSUM bank
            # per slot chunk — a start=True matmul clears has_written for
            # its WHOLE bank, so concurrently-open groups must not share a
            # bank). The last 256 d-columns are done per slot chunk after
            # the h-loop, each group alone in a rotating ps_small bank.
            yA = ps_accA.tile([128, 2048], F32, tag="accA", name="yA")
            ht_all = ht_pool.tile([128, Hc, SL], BF16)

            def emit_y512(h):
                for sc in range(SLc):
                    nc.tensor.matmul(
                        yA[:, sc * 512 : (sc + 1) * 512],
                        ht_all[:, h, ts(sc, 128)],
                        w2s[:, h, 0:512],
                        start=(h == 0),
                        stop=(h == Hc - 1),
                    )

            # Software-pipelined: y matmuls for h-1 are emitted after the
            # psh matmuls for h, so the PE runs them while ACT computes
            # gelu(h) instead of stalling on it.
            for h in range(Hc):
                psh = ps_small.tile([128, 512], F32, tag="pss", name="psH")
                for d in range(Dc):
                    nc.tensor.matmul(
                        psh[:],
                        w1s[:, d, ts(h, 128)],
                        slotsT[:, d, :],
                        start=(d == 0),
                        stop=(d == Dc - 1),
                    )
                nc.scalar.activation(
                    ht_all[:, h, :], psh[:], act_fn, bias=b1_s[:, h : h + 1]
                )
                if h > 0:
                    emit_y512(h - 1)
            emit_y512(Hc - 1)
            for sc in range(SLc):
                nc.vector.tensor_scalar_mul(
                    y_aug[:, sc, 0:512], yA[:, sc * 512 : (sc + 1) * 512], SY
                )
            for sc in range(SLc):
                pool_ = ps_small if sc % 2 == 0 else ps_q
                tag_ = "pss" if sc % 2 == 0 else "psq"
                psB = pool_.tile([128, 512], F32, tag=tag_, name="psB")
                for h in range(Hc):
                    nc.tensor.matmul(
                        psB[:, :256],
                        ht_all[:, h, ts(sc, 128)],
                        w2s[:, h, 512:768],
                        start=(h == 0),
                        stop=(h == Hc - 1),
                    )
                nc.scalar.activation(
                    y_aug[:, sc, 512:768], psB[:, :256], AF.Copy, scale=SY
                )
            # ---- phase 4: combine partials + local denominator ----
            for v in range(NV):
                ot = out_pool.tile([128, OD], BF16)
                for gi, (off, sz) in enumerate(((0, 512), (512, ODP - 512))):
                    if gi == 0:
                        pso = ps_small.tile([128, 512], F32, tag="pss", name="psO")
                    else:
                        pso = ps_q.tile([128, 512], F32, tag="psq", name="psO")
                    for sp in range(SLc // 2):
                        nc.tensor.matmul(
                            pso[:, :sz],
                            eT[:, 2 * sp : 2 * sp + 2, ts(v, 128)],
                            y_aug[:, 2 * sp : 2 * sp + 2, off : off + sz],
                            start=(sp == 0),
                            stop=(sp == SLc // 2 - 1),
                            perf_mode=DR,
                        )
                    osz = min(sz, OD - off)
                    if off == 0:
                        nc.scalar.copy(ot[:, off : off + osz], pso[:, :osz])
                    else:
                        nc.vector.tensor_copy(ot[:, off : off + osz], pso[:, :osz])
                nc.sync.dma_start(outp[b, ts(v, 128), :], ot[:])

    return nc


def _make_core_inputs(x, phi, w1, b1, w2, n_cores=N_CORES):
    B, N, Dd = x.shape
    E, S, _ = phi.shape
    H = w1.shape[2]
    halves = n_cores // E
    SL = S // halves
    Dc, Hc = Dd // 128, H // 128
    xs = SX * x
    assert np.max(np.abs(xs)) < 200.0
    xT_full = np.ascontiguousarray(xs.transpose(0, 2, 1)).reshape(
        B, Dc, 128, N
    ).astype(F8NP)
    xnb = np.ascontiguousarray(x).astype(BF)
    in_maps = []
    for c in range(n_cores):
        e, hh = c // halves, c % halves
        phi_loc = SPHI * phi[e, hh * SL : (hh + 1) * SL, :]
        assert np.max(np.abs(phi_loc)) < 200.0
        phiT = np.ascontiguousarray(phi_loc.T).reshape(Dc, 128, SL).astype(F8NP)
        in_maps.append(
            {
                "xT": xT_full,
                "xnb": xnb,
                "phiT": phiT,
                "w1b": np.ascontiguousarray(w1[e]).reshape(Dc, 128, H).astype(BF),
                "w2b": np.ascontiguousarray(w2[e]).reshape(Hc, 128, Dd).astype(BF),
                "b1": np.ascontiguousarray(b1[e]).reshape(Hc, 128),
            }
        )
    return in_maps


def _combine_core_outputs(outs, b2, n_cores=N_CORES):
    E, D = b2.shape
    halves = n_cores // E
    num = np.zeros(outs[0]["outp"][..., :D].shape, dtype=np.float64)
    den = np.zeros(outs[0]["outp"][..., D].shape, dtype=np.float64)
    num_scale = 1.0 / (2.0**SE_LOG2 * SY)
    gdl_scale = 1.0 / (2.0**SE_LOG2 * SONE)
    for c, r in enumerate(outs):
        e = c // halves
        o = r["outp"].astype(np.float64)
        gdl = o[..., D] * gdl_scale
        num += o[..., :D] * num_scale
        num += gdl[..., None] * b2[e].astype(np.float64)[None, None, :]
        den += gdl
    return (num / den[..., None]).astype(np.float32)


def _run(x, phi, w1, b1, w2, b2, trace=False, tmpdir=None):
    x = np.asarray(x, dtype=np.float32)
    phi = np.asarray(phi, dtype=np.float32)
    w1 = np.asarray(w1, dtype=np.float32)
    b1 = np.asarray(b1, dtype=np.float32)
    w2 = np.asarray(w2, dtype=np.float32)
    b2 = np.asarray(b2, dtype=np.float32)

    B, N, D = x.shape
    E, S, _ = phi.shape
    H = w1.shape[2]
    SL = S // (N_CORES // E)

    nc = bass.Bass(
        "TRN2", target_bir_lowering=False, debug=False, num_devices=N_CORES
    )
    _emit_moe_kernel(nc, B, N, D, SL, H)
    _split_excess_waits(nc)

    in_maps = _make_core_inputs(x, phi, w1, b1, w2)
    res = run_bass_kernel_spmd(
        nc, in_maps, core_ids=list(range(N_CORES)), trace=trace, tmpdir=tmpdir
    )
    return _combine_core_outputs(res.results, b2), res


def kernel(x, phi, w1, b1, w2, b2):
    return _run(x, phi, w1, b1, w2, b2)[0]
